# revision 12
# baseline (speedup 1.0000x reference)
"""Distributed Bass attention kernel for 8 TRN2 NeuronCores.

Problem: single-head causal attention, B=4, S=2048, d_model=1024, d_head=64.
  q = x@WQ.T+bq; k = x@WK.T+bk; v = x@WV.T+bv (v is d_model wide)
  out = softmax(causal(q@k.T)) @ v

Sharding: core = 2*b + half. Each core computes batch b, output channels
[half*512, (half+1)*512). Q/K/scores/softmax are duplicated within a batch
pair (cheap); V projection and attn@V are channel-split. No collectives.

Layout tricks:
  - x fed pre-transposed and bf16 (xT [d, S]) so projections contract
    d_model on partitions; q/k projections accumulate in fp32 PSUM with the
    bias folded in at eviction time (ACT Identity with per-partition bias
    AP) so no PE cycles are spent on bias.
  - scores computed transposed [keys, queries] so attn@V uses the exp'd P
    tiles directly as the stationary operand - no transposes anywhere.
  - scores matmul pads the 64-wide head dim to K=128 and uses the spare
    array rows for extra precision: lhsT = [k_hi; k_lo] (bf16 split) against
    rhs = [q_hi; q_hi] computes (k_hi + k_lo) . q_hi in one full-array pass.
  - softmax without max-subtraction (|logits| <= ~50 => exp fits fp32 fine).
    P stays unnormalized on-chip; per-query key-sums are accumulated as two
    parallel tile-sum chains (DVE + GpSimd), then reduced across the 128
    key partitions on-chip with a tiny ones^T f32r matmul per block and
    exported as a single [1, 2048] row - the host only divides.
  - the q/k projection (all 4 blocks) streams on the xt chunk DMAs and
    V-projection tiles 0-3 stream on the wv chunk DMAs right after, with
    cheap filler matmuls padding each chunk so the in-order PE queue never
    goes idle: a >~1us PE idle resets the p-state ramp to half clock for
    3us, which is far more expensive than the filler.
  - attention blocks processed in reverse size order and software-pipelined:
    block j's attn@V matmuls are interleaved into block j-1's scores/exp
    emission so the in-order PE queue never stalls (keeps the HAM clock-gate
    released); dummy warm-up matmuls cover the initial DMA window.
"""

import sys

if "/opt/trn_rl_repo" not in sys.path:
    sys.path.insert(0, "/opt/trn_rl_repo")

import numpy as np

from concourse import bacc, tile, mybir
import concourse.bass as bass
from concourse.bass_utils import run_bass_kernel_spmd

B, S, D, HD = 4, 2048, 1024, 64
N_CORES = 8
CPC = 512  # output channels per core
NCHUNK = 8  # d_model / 128

f32 = mybir.dt.float32
f32r = mybir.dt.float32r
bf16 = mybir.dt.bfloat16
AF = mybir.ActivationFunctionType
ALU = mybir.AluOpType

_cache = {}


def _build():
    nc = bacc.Bacc("TRN2", target_bir_lowering=False, debug=False, num_devices=N_CORES)

    xT = nc.dram_tensor("xT", [NCHUNK, 128, S], bf16, kind="ExternalInput")
    wqkT = nc.dram_tensor("wqkT", [128, NCHUNK, 128], bf16, kind="ExternalInput")
    bqkc = nc.dram_tensor("bqkc", [128, 1], f32, kind="ExternalInput")
    wvT = nc.dram_tensor("wvT", [NCHUNK, 128, CPC], bf16, kind="ExternalInput")
    masks = nc.dram_tensor("masks", [128, 4, 512], bf16, kind="ExternalInput")
    out = nc.dram_tensor("out", [16, 128, CPC], f32, kind="ExternalOutput")
    rs = nc.dram_tensor("rs", [1, 4 * 512], f32, kind="ExternalOutput")

    with tile.TileContext(nc) as tc:
        with (
            tc.tile_pool(name="big", bufs=1) as big,
            tc.tile_pool(name="ppool", bufs=30) as ppool,
            tc.tile_pool(name="opool", bufs=4) as opool,
            tc.tile_pool(name="small", bufs=4) as small,
            tc.tile_pool(name="spool", bufs=4) as spool,
            tc.tile_pool(name="ps_s", bufs=4, space=bass.MemorySpace.PSUM) as ps_s,
            tc.tile_pool(name="ps_v", bufs=2, space=bass.MemorySpace.PSUM) as ps_v,
            tc.tile_pool(name="ps_o", bufs=2, space=bass.MemorySpace.PSUM) as ps_o,
        ):
            # persistent SBUF tiles
            xt = big.tile([128, NCHUNK, S], bf16, tag="xt")  # 32KB/p
            wqk = big.tile([128, NCHUNK, 128], bf16, tag="wqk")  # 2KB/p
            wv = big.tile([128, NCHUNK, CPC], bf16, tag="wv")  # 8KB/p
            bqk_sb = big.tile([128, 1], f32, tag="bqk")
            mask_sb = big.tile([128, 4, 512], bf16, tag="mask")  # 4KB/p
            qkhi = big.tile([128, S], bf16, tag="qkhi")  # rows q_hi / k_hi
            khiklo = big.tile([128, S], bf16, tag="khiklo")  # [k_hi; k_lo]
            qhiqhi = big.tile([128, S], bf16, tag="qhiqhi")  # [q_hi; q_hi]
            v_sb = big.tile([128, 16, CPC], bf16, tag="v")  # 16KB/p
            ones_b = big.tile([128, 128], bf16, tag="ones_b")
            ones_c = big.tile([128, 1], f32r, tag="ones_c")
            klo_tmp = big.tile([128, 512], f32, tag="klotmp")  # rows 64:128 used
            rs_sb = big.tile([1, 4 * 512], f32, tag="rs_sb")

            # input DMAs: tiny bias + weights first, then the xt chunks
            # (q/k projection streams on them), then wv chunks (V tiles
            # 0-3 stream on those), masks last (needed mid-vproj)
            nc.sync.dma_start(out=bqk_sb[:, :], in_=bqkc[:, :])
            nc.sync.dma_start(out=wqk[:, :, :], in_=wqkT[:, :, :])
            for c in range(NCHUNK):
                nc.sync.dma_start(out=xt[:, c, :], in_=xT[c, :, :])
            for c in range(NCHUNK):
                nc.sync.dma_start(out=wv[:, c, :], in_=wvT[c, :, :])
            nc.sync.dma_start(out=mask_sb[:, :, :], in_=masks[:, :, :])
            nc.vector.memset(ones_b[:, :], 1.0)
            ones_cf = big.tile([128, 1], f32, tag="ones_cf")
            nc.gpsimd.memset(ones_cf[:, :], 1.0)
            nc.vector.tensor_copy(ones_c[:, :], ones_cf[:, :])

            # PE warmup: dummy matmuls on the ones tile while input DMA
            # streams, so the HAM clock-gate is released before real work.
            # Chained into out[0] (overwritten later) so DCE keeps them.
            warm_ps = ps_s.tile([128, 128], f32, tag="scps", name="warm_ps")
            for w in range(16):
                nc.tensor.matmul(
                    warm_ps[:, :],
                    ones_b[:, 0:128],
                    ones_b[:, 0:128],
                    start=(w == 0),
                    stop=(w == 15),
                )
            warm_sb = small.tile([128, 128], f32, tag="warm", name="warm_sb")
            nc.vector.tensor_copy(warm_sb[:, :], warm_ps[:, :])
            nc.sync.dma_start(out=out[0, :, 0:128], in_=warm_sb[:, :])

            def qk_evict(j):
                """Evict block j's q/k PSUM with bias folded in, build the
                hi/lo score operands. qk_psum[j] rows 0:64 = q, 64:128 = k."""
                blk = slice(512 * j, 512 * (j + 1))
                ps = qk_psum[j]
                # q_hi straight into its scores-rhs row block (bias added)
                nc.scalar.activation(
                    qhiqhi[0:64, blk], ps[0:64, :], AF.Identity, bias=bqk_sb[0:64, :]
                )
                # k_hi at its natural partitions (bias added)
                nc.scalar.activation(
                    qkhi[64:128, blk], ps[64:128, :], AF.Identity, bias=bqk_sb[64:128, :]
                )
                # k_lo = (k_psum + bk) - k_hi, computed as (k_psum - k_hi) + bk
                nc.vector.tensor_tensor(
                    klo_tmp[64:128, :], ps[64:128, :], qkhi[64:128, blk], ALU.subtract
                )
                nc.vector.tensor_scalar(
                    khiklo[64:128, blk],
                    klo_tmp[64:128, :],
                    bqk_sb[64:128, :],
                    None,
                    ALU.add,
                )
                # partition shifts (SBUF->SBUF DMA is the only row mover)
                nc.sync.dma_start(out=khiklo[0:64, blk], in_=qkhi[64:128, blk])
                nc.sync.dma_start(out=qhiqhi[64:128, blk], in_=qhiqhi[0:64, blk])

            # ---- streaming phase: q/k projection for all 4 blocks rides
            # the xt chunk stream. The PE consumes a chunk faster than DMA
            # delivers the next one once the p-state ramp completes, and a
            # >~1us PE idle resets the ramp to half clock for 3us - so pad
            # each chunk with cheap filler matmuls to keep the queue fed.
            qk_psum = {}
            for j in range(4):
                qk_psum[j] = ps_s.tile([128, 512], f32, tag="scps", name=f"qkps{j}")
            fill_ps = ps_o.tile([128, 128], f32, tag="ops", name="fill_ps")
            nfill = 0
            for c in range(NCHUNK):
                for j in range(4):
                    nc.tensor.matmul(
                        qk_psum[j][:, :],
                        wqk[:, c, :],
                        xt[:, c, 512 * j : 512 * (j + 1)],
                        start=(c == 0),
                        stop=(c == NCHUNK - 1),
                    )
                if 2 <= c <= 6:
                    for f in range(10):
                        nc.tensor.matmul(
                            fill_ps[:, :],
                            ones_b[:, :],
                            ones_b[:, :],
                            start=(nfill == 0),
                            stop=(c == 6 and f == 9),
                        )
                        nfill += 1
            fill_sb = small.tile([128, 128], f32, tag="warm", name="fill_sb")
            nc.vector.tensor_copy(fill_sb[:, :], fill_ps[:, :])
            nc.sync.dma_start(out=out[0, :, 128:256], in_=fill_sb[:, :])
            for j in range(4):
                qk_evict(j)

            # ---- V projection tiles 0..3 ride the wv chunk stream (two
            # PSUM accumulators from the vps pool, two from the freed
            # scps slots) ----
            v_ps4 = [
                ps_v.tile([128, CPC], f32, tag="vps", name=f"vps{t}") for t in (0, 1)
            ] + [
                ps_s.tile([128, CPC], f32, tag="scps", name=f"vps{t}") for t in (2, 3)
            ]
            for c in range(NCHUNK):
                for t in range(4):
                    nc.tensor.matmul(
                        v_ps4[t][:, :],
                        xt[:, c, 128 * t : 128 * (t + 1)],
                        wv[:, c, :],
                        start=(c == 0),
                        stop=(c == NCHUNK - 1),
                    )
            for t in range(4):
                if t % 2 == 0:
                    nc.vector.tensor_copy(v_sb[:, t, :], v_ps4[t][:, :])
                else:
                    nc.scalar.copy(v_sb[:, t, :], v_ps4[t][:, :])

            # ---- attention machinery ----
            def emit_scores(j, i, Ssum):
                # K=128 single matmul: rows 0-63 k_hi x q_hi, rows 64-127
                # k_lo x q_hi => scores = (k_hi + k_lo) . q_hi
                sc_ps = ps_s.tile([128, 512], f32, tag="scps", name=f"scps{j}_{i}")
                nc.tensor.matmul(
                    sc_ps[:, :],
                    khiklo[:, 128 * i : 128 * (i + 1)],
                    qhiqhi[:, 512 * j : 512 * (j + 1)],
                    start=True,
                    stop=True,
                )
                p = ppool.tile([128, 512], bf16, tag="p", name=f"p{j}_{i}")
                nc.scalar.activation(p[:, :], sc_ps[:, :], AF.Exp)
                if i >= 4 * j:
                    nc.vector.tensor_tensor(
                        p[:, :], p[:, :], mask_sb[:, i - 4 * j, :], ALU.mult
                    )
                eng = nc.vector if i % 2 == 0 else nc.gpsimd
                Sc = Ssum[i % 2]
                if i < 2:
                    eng.tensor_copy(Sc[:, :], p[:, :])
                else:
                    eng.tensor_tensor(Sc[:, :], Sc[:, :], p[:, :], ALU.add)
                return [p]

            def emit_export(j, Ssum):
                # reduce the two [128,512] chain tiles across key partitions
                # into one [1,512] row: ones^T @ (S0 + S1) via two f32r
                # matmuls accumulating in PSUM, then evict into rs_sb.
                ex_ps = ps_s.tile([1, 512], f32, tag="scps", name=f"exps{j}")
                for c in range(2):
                    nc.tensor.matmul(
                        ex_ps[0:1, :],
                        ones_c[:, :],
                        Ssum[c][:, :],
                        start=(c == 0),
                        stop=(c == 1),
                    )
                nc.scalar.copy(rs_sb[0:1, 512 * j : 512 * (j + 1)], ex_ps[0:1, :])

            def emit_export_dma():
                nc.sync.dma_start(out=rs[0:1, :], in_=rs_sb[0:1, :])

            def attnv_ops(j, P, reverse=False):
                ops = []
                for tq in ([3, 2, 1, 0] if reverse else range(4)):
                    t = 4 * j + tq
                    ops.append(("alloc", t))
                    for i in range(t + 1):
                        ops.append(("mm", t, i))
                    ops.append(("evac", t))
                return ops

            def emit_attnv_op(op, P, state, evac_dve=False):
                if op[0] == "alloc":
                    t = op[1]
                    state[t] = ps_o.tile([128, CPC], f32, tag="ops", name=f"ops{t}")
                elif op[0] == "mm":
                    _, t, i = op
                    nc.tensor.matmul(
                        state[t][:, :],
                        P[i][:, 128 * (t % 4) : 128 * (t % 4) + 128],
                        v_sb[:, i, :],
                        start=(i == 0),
                        stop=(i == t),
                    )
                else:
                    t = op[1]
                    o_sb = opool.tile([128, CPC], f32, tag="osb", name=f"osb{t}")
                    if evac_dve and t % 2 == 0:
                        nc.vector.tensor_copy(o_sb[:, :], state[t][:, :])
                    else:
                        nc.scalar.copy(o_sb[:, :], state[t][:, :])
                    nc.sync.dma_start(out=out[t, :, :], in_=o_sb[:, :])

            # ---- V projection tiles 2..15, with block 3's scores
            # interleaved into the tail so its exp chain (ACT) finishes
            # before attn@V needs P ----
            Ssums = {}
            Ssums[3] = [
                spool.tile([128, 512], f32r, tag=f"S{c}", name=f"S3_{c}")
                for c in range(2)
            ]
            P3 = []
            for t in range(4, 16):
                v_ps = ps_v.tile([128, CPC], f32, tag="vps")
                for c in range(NCHUNK):
                    nc.tensor.matmul(
                        v_ps[:, :],
                        xt[:, c, 128 * t : 128 * (t + 1)],
                        wv[:, c, :],
                        start=(c == 0),
                        stop=(c == NCHUNK - 1),
                    )
                # ACT is exp-saturated while block 3's scores interleave
                # (t=6..13), so keep those evictions off it
                if t % 2 == 1 and not (6 <= t < 14):
                    nc.scalar.copy(v_sb[:, t, :], v_ps[:, :])
                else:
                    nc.vector.tensor_copy(v_sb[:, t, :], v_ps[:, :])
                if 6 <= t < 14:
                    P3.extend(emit_scores(3, 2 * (t - 6), Ssums[3]))
                    P3.extend(emit_scores(3, 2 * (t - 6) + 1, Ssums[3]))

            # ---- attention blocks: scores for block j interleaved with the
            # previous (larger) block's attn@V in PE program order ----
            prev = (3, P3)  # block 3 scored during vproj; attn@V pending
            for j in [2, 1, 0, None]:
                av = attnv_ops(*prev, reverse=(j is None)) if prev is not None else []
                avP = prev[1] if prev is not None else None
                av_state = {}
                pending_export = prev[0]
                if j is None:
                    for idx, op in enumerate(av):
                        emit_attnv_op(op, avP, av_state, evac_dve=True)
                        if idx == 1 and pending_export is not None:
                            emit_export(pending_export, Ssums[pending_export])
                            emit_export_dma()
                            pending_export = None
                    break
                n = 4 * j + 4
                Ssums[j] = [
                    spool.tile([128, 512], f32r, tag=f"S{c}", name=f"S{j}_{c}")
                    for c in range(2)
                ]
                P = []
                A = list(range(n))  # score emissions
                export_at = max(2, round(0.25 * len(av)))
                # front-load a couple of score pairs, then interleave the
                # previous block's attn@V ops
                front = min(3, len(A))
                k_av = 0
                for idx, i in enumerate(A):
                    P.extend(emit_scores(j, i, Ssums[j]))
                    if idx >= front - 1:
                        want = (idx + 1 - front + 1) * len(av) / max(
                            1, len(A) - front + 1
                        )
                        while k_av < len(av) and k_av < want:
                            emit_attnv_op(av[k_av], avP, av_state)
                            k_av += 1
                            if k_av == export_at and pending_export is not None:
                                emit_export(pending_export, Ssums[pending_export])
                                pending_export = None
                while k_av < len(av):
                    emit_attnv_op(av[k_av], avP, av_state)
                    k_av += 1
                    if k_av == export_at and pending_export is not None:
                        emit_export(pending_export, Ssums[pending_export])
                        pending_export = None
                if pending_export is not None:
                    emit_export(pending_export, Ssums[pending_export])
                    pending_export = None
                prev = (j, P)

    nc.compile()
    return nc


def _get_nc():
    if "nc" not in _cache:
        _cache["nc"] = _build()
    return _cache["nc"]


def _prep_in_maps(x, WQ_w, WQ_b, WK_w, WK_b, WV_w, WV_b):
    bf = mybir.dt.np(bf16)
    wqk = np.concatenate([WQ_w, WK_w], axis=0)  # [128, D]
    wqkT = np.ascontiguousarray(
        wqk.T.reshape(NCHUNK, 128, 128).transpose(1, 0, 2)
    ).astype(bf)
    bqkc = np.concatenate([WQ_b, WK_b]).reshape(128, 1).astype(np.float32)

    # masks[m, kk, qq] = 1 if 128*m + kk <= qq else 0
    kk = np.arange(128)[:, None]
    qq = np.arange(512)[None, :]
    masks = np.ascontiguousarray(
        np.stack([(128 * m + kk <= qq) for m in range(4)], axis=0).transpose(1, 0, 2)
    ).astype(bf)

    in_maps = []
    for core in range(N_CORES):
        b, half = core // 2, core % 2
        xTb = np.ascontiguousarray(x[b].T).reshape(NCHUNK, 128, S)
        wv_sl = WV_w[half * CPC : (half + 1) * CPC]  # [CPC, D]
        wvT = np.ascontiguousarray(wv_sl.T).reshape(NCHUNK, 128, CPC)
        in_maps.append(
            {
                "xT": xTb.astype(bf),
                "wqkT": wqkT,
                "bqkc": bqkc,
                "wvT": wvT.astype(bf),
                "masks": masks,
            }
        )
    return in_maps


def _run(in_maps, trace=False, **kw):
    nc = _get_nc()
    return run_bass_kernel_spmd(
        nc, in_maps, core_ids=list(range(N_CORES)), trace=trace, **kw
    )


def kernel(x, WQ_w, WQ_b, WK_w, WK_b, WV_w, WV_b):
    x = np.asarray(x, dtype=np.float32)
    in_maps = _prep_in_maps(
        x,
        np.asarray(WQ_w, np.float32),
        np.asarray(WQ_b, np.float32),
        np.asarray(WK_w, np.float32),
        np.asarray(WK_b, np.float32),
        np.asarray(WV_w, np.float32),
        np.asarray(WV_b, np.float32),
    )
    res = _run(in_maps, trace=False)
    out = np.empty((B, S, D), dtype=np.float32)
    for core in range(N_CORES):
        b, half = core // 2, core % 2
        shard = res.results[core]["out"].reshape(S, CPC)
        if half == 0:
            den = res.results[core]["rs"].reshape(S)
            out[b] = 0.0
        out[b, :, half * CPC : (half + 1) * CPC] = shard
        if half == 1:
            out[b] /= den[:, None]
    out += np.asarray(WV_b, np.float32)[None, None, :]
    return out
